# revision 14
# baseline (speedup 1.0000x reference)
"""COMA loss kernel for Trainium2 — v4: N-on-partition + j-major free dim.

Layout per core (B sharded 8 ways, BL=16, BA=BL*A=128 rows):
  ba = 64*h + j  (h in {0,1}, j in [0,64))
  SBUF partition p = 64*h + n   (n = action index, N=64)
  free index     f = j*T + t    (F = 64*T = 16384), j-major!

All six per-(ba,t) sums over n are PE ones-matmuls (accumulating six
[128,12] one-column stationaries into one [12,512] PSUM tile). Because
f is j-major, the [12, F] sum rows convert to the stage-2 layout
s2d[j, 12, T] with a single strided SBUF->SBUF DMA per chunk (512B
contiguous segments) — no PE transposes at all. Stage-1 streams over
j-blocks of 8 (chunks are f-contiguous). The onehot is a 4x-mode
tensor_scalar is_equal against a per-partition iota. DVE carries only
the five fp16 products plus a compact merged-h stage 2.
"""

import sys

for _p in ("/opt/trn_rl_repo",):
    if _p not in sys.path:
        sys.path.insert(0, _p)

import numpy as np

import concourse.bass as bass
import concourse.bacc as bacc
import concourse.mybir as mybir
from concourse.bass_utils import run_bass_kernel_spmd
from concourse.tile import TileContext

T, B, A, N = 256, 128, 8, 64
M = 8                 # cores
BL = B // M
BA = BL * A           # 128
H, J = 2, 64          # ba = 64h + j
F = J * T             # 16384: f = j*T + t
JCH = 8               # j per chunk
NCH = J // JCH        # 8 chunks
FCH = JCH * T         # 2048
SUB = 512             # matmul f-subchunk (one PSUM bank)
NSUB = FCH // SUB     # 4
GAMMA, LAMBDA = 0.99, 0.95

F32 = mybir.dt.float32
F16 = mybir.dt.float16

# reduction-row order within [12, f]: row = 2*rho + h
R_SUME, R_DOTEQ, R_DOTEL, R_QTK, R_TQTK, R_LTK = range(6)


def build_program() -> bass.Bass:
    nc = bacc.Bacc("TRN2", target_bir_lowering=False, debug=False)

    lg_d = nc.dram_tensor("logit", [BA, F], F16, kind="ExternalInput")
    qv_d = nc.dram_tensor("qv", [BA, F], F16, kind="ExternalInput")
    tqv_d = nc.dram_tensor("tqv", [BA, F], F16, kind="ExternalInput")
    actr0_d = nc.dram_tensor("actr0", [BA, FCH], F16, kind="ExternalInput")
    actr1_d = nc.dram_tensor("actr1", [BA, F - FCH], F16, kind="ExternalInput")
    iota_d = nc.dram_tensor("iotac", [BA, 1], F32, kind="ExternalInput")
    wred_d = nc.dram_tensor("wred", [BA, 6 * 12], F16, kind="ExternalInput")
    wgt_d = nc.dram_tensor("wgt", [J, H * T], F16, kind="ExternalInput")
    rwd_d = nc.dram_tensor("rwd", [J, H * T], F16, kind="ExternalInput")
    out_d = nc.dram_tensor("out", [J, 3], F32, kind="ExternalOutput")

    OP = mybir.AluOpType
    AX = mybir.AxisListType.X

    with TileContext(nc) as tc:
        with (
            tc.tile_pool(name="inp", bufs=3) as inp,
            tc.tile_pool(name="scr", bufs=3) as scr,
            tc.tile_pool(name="sums", bufs=8) as sums,
            tc.tile_pool(name="per", bufs=1) as per,
            tc.tile_pool(name="ps_red", bufs=6, space=bass.MemorySpace.PSUM) as ps_red,
            tc.tile_pool(name="drb", bufs=8, space="DRAM") as drb,
        ):
            # ---- constants / small inputs ---------------------------------
            iota_c = per.tile([BA, 1], F32)
            nc.sync.dma_start(out=iota_c[:], in_=iota_d[:])
            wred = per.tile([BA, 6, 12], F16)
            nc.sync.dma_start(out=wred[:], in_=wred_d[:])
            w_t = per.tile([J, H, T], F16)
            nc.sync.dma_start(out=w_t[:], in_=wgt_d[:])
            r_t = per.tile([J, H, T], F16)
            nc.sync.dma_start(out=r_t[:], in_=rwd_d[:])
            act_rep0 = per.tile([BA, FCH], F16)
            nc.sync.dma_start(out=act_rep0[:], in_=actr0_d[:])
            act_rep1 = per.tile([BA, F - FCH], F16)

            # s2d[j, r, t]: per-(ba,t) sums in stage-2 layout, r = 2*rho+h
            s2d = per.tile([J, 12, T], F16)

            # ---- stage 1: stream j-chunks ---------------------------------
            sums_tiles = []
            for c in range(NCH):
                fsl = slice(c * FCH, (c + 1) * FCH)

                lg = inp.tile([BA, FCH], F16, tag="lg")
                qt = inp.tile([BA, FCH], F16, tag="qt")
                tq = inp.tile([BA, FCH], F16, tag="tq")
                nc.sync.dma_start(out=lg[:], in_=lg_d[:, fsl])
                nc.sync.dma_start(out=qt[:], in_=qv_d[:, fsl])
                nc.sync.dma_start(out=tq[:], in_=tqv_d[:, fsl])
                if c == 1:
                    # bulk of the replicated-action tensor loads after the
                    # second chunk's tensors; chunks 0-1 compute on their
                    # own slices while it streams
                    nc.sync.dma_start(out=act_rep1[:], in_=actr1_d[:])

                e = scr.tile([BA, FCH], F16, tag="e")
                nc.scalar.activation(
                    out=e[:], in_=lg[:], func=mybir.ActivationFunctionType.Exp
                )

                peq = scr.tile([BA, FCH], F16, tag="peq")
                nc.vector.tensor_mul(peq[:], e[:], qt[:])
                pel = scr.tile([BA, FCH], F16, tag="pel")
                nc.vector.tensor_mul(pel[:], e[:], lg[:])

                # onehot over the partition-resident n: 4x tensor_scalar
                oh = scr.tile([BA, FCH], F16, tag="oh")
                nc.vector.tensor_scalar(
                    out=oh[:],
                    in0=(
                        act_rep0[:]
                        if c == 0
                        else act_rep1[:, (c - 1) * FCH : c * FCH]
                    ),
                    scalar1=iota_c[:],
                    scalar2=None,
                    op0=OP.is_equal,
                )

                gq = scr.tile([BA, FCH], F16, tag="gq")
                nc.vector.tensor_mul(gq[:], oh[:], qt[:])
                gtq = scr.tile([BA, FCH], F16, tag="gtq")
                nc.vector.tensor_mul(gtq[:], oh[:], tq[:])
                glg = scr.tile([BA, FCH], F16, tag="glg")
                nc.vector.tensor_mul(glg[:], oh[:], lg[:])

                # six reductions accumulate into one [12, SUB] PSUM tile:
                # stationary W_rho[p, m] = 1 iff m == 2*rho + h(p).
                sums_c = sums.tile([12, JCH, T], F16, tag="sums")
                prods = [
                    (R_SUME, e),
                    (R_DOTEQ, peq),
                    (R_DOTEL, pel),
                    (R_QTK, gq),
                    (R_TQTK, gtq),
                    (R_LTK, glg),
                ]
                jps = SUB // T  # j's per 512-subchunk
                for s in range(NSUB):
                    ssl = slice(s * SUB, (s + 1) * SUB)
                    ps = ps_red.tile([12, jps, T], F32, tag="red")
                    for i, (rho, p) in enumerate(prods):
                        nc.tensor.matmul(
                            out=ps[:],
                            lhsT=wred[:, rho, :],
                            rhs=p[:, ssl],
                            start=(i == 0),
                            stop=(i == len(prods) - 1),
                        )
                    nc.scalar.activation(
                        out=sums_c[:, s * jps : (s + 1) * jps, :],
                        in_=ps[:],
                        func=mybir.ActivationFunctionType.Copy,
                    )

                sums_tiles.append(sums_c)

            # repack [12, (j, t)] -> s2d[j, 12, t] via DRAM bounces, all
            # deferred here so the stream loop's loads own the DMA rings
            # (a single DMA cannot swap the partition axis with a free
            # axis between two SBUF tiles; DRAM APs are free-form)
            for c, sums_c in enumerate(sums_tiles):
                sc = drb.tile([12, JCH, T], F16, tag=f"sc{c}")
                nc.sync.dma_start(out=sc[:], in_=sums_c[:])
                nc.sync.dma_start(
                    out=s2d[c * JCH : (c + 1) * JCH, :, :],
                    in_=sc[:].transpose([1, 0, 2]),
                )

            # ---- stage 2: merged-h ops on [J, 2, T] slices ----------------
            def S(rho):
                return s2d[:, 2 * rho : 2 * rho + 2, :]

            # lambda returns per half first: independent of the z/rs chain
            d = per.tile([J, H, T - 1], F16)
            nc.vector.tensor_scalar_mul(
                d[:], S(R_TQTK)[:, :, 1:T], GAMMA * (1.0 - LAMBDA)
            )
            nc.vector.tensor_add(d[:], d[:], r_t[:, :, 0 : T - 1])
            gl = per.tile([J, 1], F16)
            nc.vector.memset(gl[:], GAMMA * LAMBDA)
            ret = per.tile([J, H, T - 1], F16)
            for h in range(H):
                nc.vector.tensor_tensor_scan(
                    out=ret[:, h, ::-1],
                    data0=gl[:].to_broadcast([J, T - 1]),
                    data1=d[:, h, ::-1],
                    initial=s2d[:, 2 * R_TQTK + h, T - 1 : T],
                    op0=OP.mult,
                    op1=OP.add,
                )

            z = per.tile([J, H, T], F16)
            nc.scalar.activation(
                out=z[:], in_=S(R_SUME), func=mybir.ActivationFunctionType.Ln
            )
            se32 = per.tile([J, H, T], F32)
            nc.vector.tensor_copy(se32[:], S(R_SUME))
            rs = per.tile([J, H, T], F32)
            nc.vector.reciprocal_approx_fast(rs[:], se32[:])

            logp = per.tile([J, H, T], F16)
            nc.vector.tensor_tensor(out=logp[:], in0=S(R_LTK), in1=z[:], op=OP.subtract)
            bl = per.tile([J, H, T], F16)
            nc.vector.tensor_mul(bl[:], S(R_DOTEQ), rs[:])
            adv = per.tile([J, H, T], F16)
            nc.vector.tensor_tensor(out=adv[:], in0=S(R_QTK), in1=bl[:], op=OP.subtract)
            ent = per.tile([J, H, T], F16)
            nc.vector.tensor_mul(ent[:], S(R_DOTEL), rs[:])
            nc.vector.tensor_tensor(out=ent[:], in0=z[:], in1=ent[:], op=OP.subtract)

            pol = per.tile([J, H, T], F16)
            nc.vector.tensor_mul(pol[:], logp[:], adv[:])
            nc.vector.tensor_mul(pol[:], pol[:], w_t[:])
            entw = per.tile([J, H, T], F16)
            nc.vector.tensor_mul(entw[:], ent[:], w_t[:])

            qd = per.tile([J, H, T - 1], F16)
            nc.vector.tensor_tensor(
                out=qd[:], in0=ret[:], in1=S(R_QTK)[:, :, 0 : T - 1], op=OP.subtract
            )
            nc.vector.tensor_mul(qd[:], qd[:], qd[:])
            nc.vector.tensor_mul(qd[:], qd[:], w_t[:, :, 0 : T - 1])

            partials = per.tile([J, 3], F32)
            dump = per.tile([J, H, T], F16)
            nc.scalar.activation(
                out=dump[:], in_=pol[:],
                func=mybir.ActivationFunctionType.Copy,
                accum_out=partials[:, 0:1],
            )
            nc.scalar.activation(
                out=dump[:, :, 0 : T - 1], in_=qd[:],
                func=mybir.ActivationFunctionType.Copy,
                accum_out=partials[:, 1:2],
            )
            nc.scalar.activation(
                out=dump[:], in_=entw[:],
                func=mybir.ActivationFunctionType.Copy,
                accum_out=partials[:, 2:3],
            )
            nc.sync.dma_start(out=out_d[:], in_=partials[:])

    return nc


def make_in_maps(logit, action, q_value, target_q_value, reward, weight):
    """Shard + marshal full inputs into per-core input dicts."""
    logit = np.asarray(logit, np.float32)
    q_value = np.asarray(q_value, np.float32)
    target_q_value = np.asarray(target_q_value, np.float32)
    action = np.asarray(action)
    reward = np.asarray(reward, np.float32)
    weight = np.asarray(weight, np.float32)

    iota_c = (np.arange(BA, dtype=np.float32) % J).reshape(BA, 1)
    wred = np.zeros((BA, 6, 12), np.float16)
    for rho in range(6):
        wred[:J, rho, 2 * rho] = 1.0
        wred[J:, rho, 2 * rho + 1] = 1.0
    wred = wred.reshape(BA, 72)

    in_maps = []
    for r in range(M):
        bs, be = r * BL, (r + 1) * BL

        def big(x):
            # [T, BL, A, N] = [t, (h,j), n] -> [h, n, j, t] -> [128, F]
            y = x[:, bs:be].reshape(T, 2, J, N).transpose(1, 3, 2, 0)
            return np.ascontiguousarray(y).reshape(BA, F).astype(np.float16)

        act_c = action[:, bs:be].reshape(T, 2, J)  # [t, h, j]
        # act_rep[64h+n, j*T+t] = action[t, 64h+j]
        act_rep = np.ascontiguousarray(
            np.broadcast_to(
                act_c.transpose(1, 2, 0)[:, None, :, :], (2, N, J, T)
            )
        ).reshape(BA, F).astype(np.float16)

        def small(x):
            # [T, 128] -> [j, h, t]
            y = x.reshape(T, 2, J).transpose(2, 1, 0)
            return np.ascontiguousarray(y).reshape(J, H * T).astype(np.float16)

        in_maps.append(
            {
                "logit": big(logit),
                "qv": big(q_value),
                "tqv": big(target_q_value),
                "actr0": act_rep[:, :FCH].copy(),
                "actr1": act_rep[:, FCH:].copy(),
                "iotac": iota_c,
                "wred": wred,
                "wgt": small(weight[:, bs:be].reshape(T, BA)),
                "rwd": small(np.repeat(reward[:, bs:be], A, axis=1)),
            }
        )
    return in_maps


def combine_partials(partials_per_core):
    """[M][64, 6] partial sums -> the three scalar losses."""
    s = np.stack(partials_per_core).astype(np.float64).sum(axis=(0, 1))
    pol, qd, ent = s[0], s[1], s[2]
    policy_loss = np.float32(-pol / (T * B * A))
    q_value_loss = np.float32(qd / ((T - 1) * B * A))
    entropy_loss = np.float32(ent / (T * B * A))
    return policy_loss, q_value_loss, entropy_loss


_program_cache = {}


def _get_program() -> bass.Bass:
    if "nc" not in _program_cache:
        nc = build_program()
        nc.finalize()
        _program_cache["nc"] = nc
    return _program_cache["nc"]


def kernel(logit, action, q_value, target_q_value, reward, weight):
    nc = _get_program()
    in_maps = make_in_maps(logit, action, q_value, target_q_value, reward, weight)
    res = run_bass_kernel_spmd(nc, in_maps, list(range(M))).results
    return combine_partials(
        [np.asarray(res[i]["out"]).reshape(J, 3) for i in range(M)]
    )
